# revision 5
# baseline (speedup 1.0000x reference)
"""GC-LSTM (graph-conv LSTM) Trainium2 kernel — v2 (block-diagonal layout).

Model (per batch b, timestep t):
    msg  = relu([x_t, h] @ Wg + bg)          # [N, H]
    agg  = adj @ msg                         # graph aggregation over stations
    gates= agg @ Wl + bl                     # [N, 4H]
    c    = sig(f)*c + sig(i)*tanh(g)
    h    = sig(o)*tanh(c)
final: out = relu(h[:, 0, :] @ Wr1 + br1) @ Wr2 + br2     # [B, 1]

Sharding: data-parallel over B=32 across 8 cores (BL=4 per core), adj and
weights replicated, h/c device-local through the T=168 scan.

v2 design notes (vs baseline): minimize PE instruction count and columns
by padding batches into the contraction dim with block-diagonal weights:
  M1x: lhsT = stacked x of 4 batches [36,128], rhs = 4-blockdiag Wgx
       [36,256] -> 4 matmuls/step (out [n-chunk,(b,h')])
  M1h: lhsT = hT2 slice [128=(q,h),128], rhs = 2-blockdiag Wgh [128,128]
       -> 8 matmuls/step, accumulating into the same PSUM regions as M1x
  M2:  unchanged orientation: aggT[(q,h'),m] per pair, 4 k-chunks
  M3:  lhsT = 2-blockdiag Wl_gate [128,128], rhs = agg_sb [128,512]
       -> 4 matmuls/pair with full-width [128,512] outputs per gate
Gate PSUM tiles per pair: t_if [128,(i m|f m)], t_og [128,(o m|g m)];
activations stay full-partition-width. relu on DVE, sig/tanh on ACT,
LSTM arithmetic split DVE/GpSimd.

Layouts:
  hT2/cT2 [128=(q,h), (pair,n)=1024]   (q = batch parity in pair)
  msg_ps  [128=n-chunk, (k, b, h')=1024]  b-major 256 per k-chunk
  msg_sb  same, fp16
  agg_sb  [128=(q,h), m=512] per pair, fp16
"""

import sys
import types
from contextlib import ExitStack

import numpy as np

import concourse.bass as bass
import concourse.mybir as mybir
import concourse.tile as tile
from concourse import bacc
from concourse.bass_utils import run_bass_kernel_spmd


def _ensure_ntff_hook():
    """Register the axon NTFF profile hook if the image's antenv lacks it."""
    try:
        from antenv import axon_hooks  # noqa: F401
        return
    except ImportError:
        pass
    hook = None
    try:
        import importlib
        tb = importlib.import_module("trn_agent_boot.trn_boot")
        hook = tb._ntff_profile_via_ctypes("/opt/axon/libaxon_pjrt.so")
    except Exception:
        hook = None
    mod = types.ModuleType("antenv.axon_hooks")
    mod._hook = hook
    mod.set_axon_ntff_profile_hook = lambda h: setattr(mod, "_hook", h)
    mod.get_axon_ntff_profile_hook = lambda: mod._hook
    import antenv
    sys.modules["antenv.axon_hooks"] = mod
    antenv.axon_hooks = mod


AF = mybir.ActivationFunctionType
DT = mybir.dt
ALU = mybir.AluOpType

# Problem constants (hardcoded per contract)
B, N, T, F = 32, 512, 168, 8
H = 64
NCORES = 8
BL = B // NCORES          # 4 batches per core
NPAIR = BL // 2           # 2 pairs
KT = N // 128             # 4 station k-tiles
XROWS = BL * (F + 1)      # 36: stacked (x, ones) rows for 4 batches
TCH = 8                   # timesteps per x DMA chunk


def build_program(t_steps: int = T, n_devices: int = NCORES):
    nc = bacc.Bacc("TRN2", target_bir_lowering=False, debug=False,
                   num_devices=n_devices)

    fp32 = DT.float32
    f16 = DT.float16

    # DRAM I/O (per-core values supplied via in_maps)
    xT4_d = nc.dram_tensor("xT4", [XROWS, t_steps, N], f16, kind="ExternalInput").ap()
    adjT_d = nc.dram_tensor("adjT", [128, KT * N], f16, kind="ExternalInput").ap()
    wgx4_d = nc.dram_tensor("wgx4", [XROWS, BL * H], f16, kind="ExternalInput").ap()
    wghbd_d = nc.dram_tensor("wghbd", [128, 128], f16, kind="ExternalInput").ap()
    wlbd_d = nc.dram_tensor("wlbd", [128, 4 * 128], f16, kind="ExternalInput").ap()
    wr12_d = nc.dram_tensor("wr12", [128, H // 2], f16, kind="ExternalInput").ap()
    br1_d = nc.dram_tensor("br1", [H // 2, 1], fp32, kind="ExternalInput").ap()
    wr2_d = nc.dram_tensor("wr2", [H // 2, 1], f16, kind="ExternalInput").ap()
    br2_d = nc.dram_tensor("br2", [1, 1], fp32, kind="ExternalInput").ap()
    out_d = nc.dram_tensor("out", [1, BL], fp32, kind="ExternalOutput").ap()

    n_chunks = (t_steps + TCH - 1) // TCH

    with tile.TileContext(nc) as tc, ExitStack() as ctx:
        const_pool = ctx.enter_context(tc.tile_pool(name="const", bufs=1))
        state_pool = ctx.enter_context(tc.tile_pool(name="state", bufs=1))
        xc_pool = ctx.enter_context(tc.tile_pool(name="xc", bufs=2))
        msgs_pool = ctx.enter_context(tc.tile_pool(name="msgs", bufs=2))
        aggs_pool = ctx.enter_context(tc.tile_pool(name="aggs", bufs=2))
        sif_pool = ctx.enter_context(tc.tile_pool(name="sif", bufs=2))
        sotg_pool = ctx.enter_context(tc.tile_pool(name="sotg", bufs=4))
        tmp_pool = ctx.enter_context(tc.tile_pool(name="tmp", bufs=6))

        pp_msg = ctx.enter_context(tc.tile_pool(name="pp_msg", bufs=1, space="PSUM"))
        pp_agg = ctx.enter_context(tc.tile_pool(name="pp_agg", bufs=2, space="PSUM"))
        pp_gate = ctx.enter_context(tc.tile_pool(name="pp_gate", bufs=1, space="PSUM"))

        # ---- constants ----
        adjT_sb = const_pool.tile([128, KT * N], f16)
        nc.sync.dma_start(adjT_sb[:], adjT_d[:])
        wgx4_sb = const_pool.tile([XROWS, BL * H], f16)
        nc.sync.dma_start(wgx4_sb[:], wgx4_d[:])
        wghbd_sb = const_pool.tile([128, 128], f16)
        nc.sync.dma_start(wghbd_sb[:], wghbd_d[:])
        wlbd_sb = const_pool.tile([128, 4 * 128], f16)
        nc.sync.dma_start(wlbd_sb[:], wlbd_d[:])
        wr12_sb = const_pool.tile([128, H // 2], f16)
        nc.sync.dma_start(wr12_sb[:], wr12_d[:])
        br1_sb = const_pool.tile([H // 2, 1], fp32)
        nc.sync.dma_start(br1_sb[:], br1_d[:])
        wr2_sb = const_pool.tile([H // 2, 1], f16)
        nc.sync.dma_start(wr2_sb[:], wr2_d[:])
        br2_sb = const_pool.tile([1, 1], fp32)
        nc.sync.dma_start(br2_sb[:], br2_d[:])

        # ---- persistent state, zero-init ----
        hT2 = state_pool.tile([128, NPAIR * N], f16)
        cT2 = state_pool.tile([128, NPAIR * N], fp32)
        nc.gpsimd.memset(hT2[:], 0.0)
        nc.gpsimd.memset(cT2[:], 0.0)

        # prefetch first x chunk
        xc_tiles = {}
        xc_tiles[0] = xc_pool.tile([XROWS, TCH * N], f16, tag="xc", name="xc_t0")
        nc.sync.dma_start(
            xc_tiles[0][:].rearrange("p (t n) -> p t n", t=TCH, n=N),
            xT4_d[:, 0:TCH, :])

        for t in range(t_steps):
            ch, off = divmod(t, TCH)
            off *= N
            if t % TCH == 0 and (ch + 1) < n_chunks:
                nt = min(TCH, t_steps - (ch + 1) * TCH)
                xc_tiles[ch + 1] = xc_pool.tile([XROWS, TCH * N], f16, tag="xc",
                                                name=f"xc_t{ch + 1}")
                nc.sync.dma_start(
                    xc_tiles[ch + 1][:, 0:nt * N].rearrange(
                        "p (t n) -> p t n", t=nt, n=N),
                    xT4_d[:, (ch + 1) * TCH:(ch + 1) * TCH + nt, :])
                if ch - 1 in xc_tiles:
                    del xc_tiles[ch - 1]
            xc = xc_tiles[ch]

            # ---- M1: msg = relu(x Wgx + bg + h Wgh) ----
            # NOTE: accumulation-group matmuls for one PSUM region must be
            # emitted consecutively (x start, then the pair h stops) — see
            # repro_psum.py: splitting starts and stops zeroes regions.
            msg_ps = pp_msg.tile([128, KT * BL * H], fp32, tag="msg_ps")
            for k in range(KT):
                nc.tensor.matmul(
                    msg_ps[:, k * 256:(k + 1) * 256],
                    xc[:, off + k * 128: off + (k + 1) * 128],
                    wgx4_sb[:],
                    start=True, stop=False, skip_group_check=True,
                )
                for pair in range(NPAIR):
                    nc.tensor.matmul(
                        msg_ps[:, k * 256 + pair * 128: k * 256 + (pair + 1) * 128],
                        hT2[:, pair * N + k * 128: pair * N + (k + 1) * 128],
                        wghbd_sb[:],
                        start=False, stop=True, skip_group_check=True,
                    )

            # relu (DVE), per pair: strided view over the 4 k-chunks
            msg_sb = msgs_pool.tile([128, KT * BL * H], f16, tag="msg_sb")
            msg_ps_v = msg_ps[:].rearrange("p (k pb c) -> p k pb c",
                                           k=KT, pb=NPAIR, c=128)
            msg_sb_v = msg_sb[:].rearrange("p (k pb c) -> p k pb c",
                                           k=KT, pb=NPAIR, c=128)
            for pair in range(NPAIR):
                nc.vector.tensor_scalar_max(
                    msg_sb_v[:, :, pair:pair + 1, :],
                    msg_ps_v[:, :, pair:pair + 1, :],
                    0.0,
                )

            for pair in range(NPAIR):
                # ---- M2: aggT_pair = msg_pair^T @ adjT ----
                agg_ps = pp_agg.tile([128, N], fp32, tag="agg_ps")
                for k in range(KT):
                    nc.tensor.matmul(
                        agg_ps[:],
                        msg_sb[:, k * 256 + pair * 128: k * 256 + (pair + 1) * 128],
                        adjT_sb[:, k * N:(k + 1) * N],
                        start=(k == 0), stop=(k == KT - 1),
                    )
                agg_sb = aggs_pool.tile([128, N], f16, tag="agg_sb")
                nc.vector.tensor_copy(agg_sb[:], agg_ps[:])

                # ---- M3: gates, blockdiag Wl; gate order (i, f, o, g) ----
                t_if = pp_gate.tile([128, 2 * N], fp32, tag="t_if")
                t_og = pp_gate.tile([128, 2 * N], fp32, tag="t_og")
                for gi, dst in ((0, t_if[:, 0:N]), (1, t_if[:, N:2 * N]),
                                (2, t_og[:, 0:N]), (3, t_og[:, N:2 * N])):
                    nc.tensor.matmul(
                        dst, wlbd_sb[:, gi * 128:(gi + 1) * 128], agg_sb[:],
                        start=True, stop=True,
                    )

                # ---- activations (ACT) ----
                tg = sotg_pool.tile([128, N], f16, tag="tg")
                nc.scalar.activation(tg[:], t_og[:, N:2 * N], AF.Tanh)
                sif = sif_pool.tile([128, 2 * N], f16, tag="sif")
                nc.scalar.activation(sif[:], t_if[:], AF.Sigmoid)
                so = sotg_pool.tile([128, N], f16, tag="so")
                nc.scalar.activation(so[:], t_og[:, 0:N], AF.Sigmoid)

                # ---- LSTM state update ----
                c_sl = cT2[:, pair * N:(pair + 1) * N]
                t2 = tmp_pool.tile([128, N], f16, tag="t2")
                nc.gpsimd.tensor_tensor(t2[:], sif[:, 0:N], tg[:], ALU.mult)
                t1 = tmp_pool.tile([128, N], fp32, tag="t1")
                nc.vector.tensor_tensor(t1[:], sif[:, N:2 * N], c_sl, ALU.mult)
                nc.vector.tensor_tensor(c_sl, t1[:], t2[:], ALU.add)
                tc_ = tmp_pool.tile([128, N], f16, tag="tc")
                nc.scalar.activation(tc_[:], c_sl, AF.Tanh)
                nc.vector.tensor_tensor(hT2[:, pair * N:(pair + 1) * N],
                                        so[:], tc_[:], ALU.mult)

        # ---- readout head: feat = h[:, station 0, :] ----
        r1_ps = pp_agg.tile([H // 2, BL], fp32, tag="agg_ps")
        for b in range(BL):
            pair, q = divmod(b, 2)
            nc.tensor.matmul(
                r1_ps[:, b:b + 1],
                wr12_sb[q * H:(q + 1) * H, :],
                hT2[q * H:(q + 1) * H, pair * N: pair * N + 1],
                start=True, stop=True,
            )
        r1_sb = tmp_pool.tile([H // 2, BL], f16, tag="r1")
        nc.scalar.activation(r1_sb[:], r1_ps[:], AF.Relu, bias=br1_sb[:])
        r2_ps = pp_agg.tile([1, BL], fp32, tag="agg_ps")
        nc.tensor.matmul(r2_ps[:], wr2_sb[:], r1_sb[:], start=True, stop=True)
        out_sb = tmp_pool.tile([1, BL], fp32, tag="out")
        nc.scalar.activation(out_sb[:], r2_ps[:], AF.Identity, bias=br2_sb[:])
        nc.sync.dma_start(out_d[:], out_sb[:])

    nc.compile()
    return nc


def prep_inputs(pollution_seq, adj, Wg, bg, Wl, bl, Wr1, br1, Wr2, br2,
                t_steps: int = T):
    """Host-side prep: shard + relayout. Returns per-core in_maps list."""
    assert np.allclose(bl, 0.0), "kernel folds bl only for bl==0"
    f32, f16 = np.float32, np.float16

    adjT = np.ascontiguousarray(adj.T).astype(f32)          # [n, m]
    adjT_tiled = np.ascontiguousarray(
        adjT.reshape(KT, 128, N).transpose(1, 0, 2).reshape(128, KT * N)
    ).astype(f16)

    wgx = Wg[:F].astype(f32)                                 # [8, 64]
    wgh = Wg[F:].astype(f32)                                 # [64, 64]

    wgx4 = np.zeros((XROWS, BL * H), f32)
    for b in range(BL):
        wgx4[b * (F + 1):b * (F + 1) + F, b * H:(b + 1) * H] = wgx
        wgx4[b * (F + 1) + F, b * H:(b + 1) * H] = bg
    wghbd = np.zeros((128, 128), f32)
    wghbd[0:H, 0:H] = wgh
    wghbd[H:128, H:128] = wgh
    # gate order (i, f, o, g) <- reference (i, f, g, o)
    wlbd = np.zeros((128, 4 * 128), f32)
    for gi, gc in enumerate((0, 1, 3, 2)):
        blk = Wl[:, gc * H:(gc + 1) * H]
        wlbd[0:H, gi * 128:gi * 128 + H] = blk
        wlbd[H:128, gi * 128 + H:(gi + 1) * 128] = blk

    wr12 = np.concatenate([Wr1, Wr1], axis=0)                # [128, 32]

    common = dict(
        adjT=adjT_tiled, wgx4=wgx4.astype(f16), wghbd=wghbd.astype(f16),
        wlbd=wlbd.astype(f16), wr12=wr12.astype(f16),
        br1=br1.reshape(H // 2, 1).astype(f32),
        wr2=Wr2.reshape(H // 2, 1).astype(f16),
        br2=br2.reshape(1, 1).astype(f32),
    )

    in_maps = []
    for i in range(NCORES):
        xc = pollution_seq[i * BL:(i + 1) * BL, :, :t_steps, :]  # [4, 512, t, 8]
        xT = xc.transpose(0, 3, 2, 1)                            # [4, 8, t, 512]
        ones = np.ones((BL, 1, t_steps, N), f32)
        xT4 = np.concatenate([xT.astype(f32), ones], axis=1)     # [4, 9, t, 512]
        xT4 = xT4.reshape(XROWS, t_steps, N)
        m = dict(common)
        m["xT4"] = np.ascontiguousarray(xT4).astype(f16)
        in_maps.append(m)
    return in_maps


_NC_CACHE = {}


def _get_program(t_steps):
    if t_steps not in _NC_CACHE:
        _NC_CACHE[t_steps] = build_program(t_steps)
    return _NC_CACHE[t_steps]


def kernel(pollution_seq, adj, Wg, bg, Wl, bl, Wr1, br1, Wr2, br2,
           trace=False):
    if trace:
        _ensure_ntff_hook()
    nc = _get_program(T)
    in_maps = prep_inputs(pollution_seq, adj, Wg, bg, Wl, bl, Wr1, br1,
                          Wr2, br2, T)
    res = run_bass_kernel_spmd(nc, in_maps, list(range(NCORES)), trace=trace)
    outs = [res.results[i]["out"].reshape(1, BL) for i in range(NCORES)]
    full = np.concatenate([o.T for o in outs], axis=0).astype(np.float32)  # [32,1]
    if trace:
        kernel.last_exec_time_ns = res.exec_time_ns
        kernel.last_results = res
    return full


# revision 8
# speedup vs baseline: 1.0801x; 1.0801x over previous
"""GC-LSTM (graph-conv LSTM) Trainium2 kernel — v3 (staggered pair pipeline).

Model (per batch b, timestep t):
    msg  = relu([x_t, h] @ Wg + bg)          # [N, H]
    agg  = adj @ msg                         # graph aggregation over stations
    gates= agg @ Wl + bl                     # [N, 4H]
    c    = sig(f)*c + sig(i)*tanh(g)
    h    = sig(o)*tanh(c)
final: out = relu(h[:, 0, :] @ Wr1 + br1) @ Wr2 + br2     # [B, 1]

Sharding: data-parallel over B=32 across 8 cores (BL=4 per core), adj and
weights replicated, h/c device-local through the T=168 scan.

v3 design:
- 4 batches/core processed as 2 pairs; each pair's recurrence chain is a
  software pipeline stage, emitted as alternating half-step blocks
  (p0@t, p1@t, p0@t+1, ...) so one pair's PE work overlaps the other
  pair's activation/LSTM tail.
- Block-diagonal weights put both batches of a pair in one matmul:
    M1x: lhsT = stacked x [18,128], rhs = 2-blockdiag Wgx [18,128]
    M1h: lhsT = hT2 slice [128,128], rhs = 2-blockdiag Wgh [128,128]
    M3:  lhsT = 2-blockdiag Wl_gate [128,128], rhs = agg_sb [128,512]
  (accumulation-group matmuls for one PSUM region stay consecutive).
- Gates per pair: t_ifo [128,1536] PSUM (one full-width sigmoid) and
  t_g [128,512] (one tanh). relu & agg cast on GpSimd; LSTM arithmetic
  on DVE; sig/tanh on ACT.

Layouts:
  hT2/cT2 [128=(q,h), (pair,n)=1024]  (q = batch parity in pair)
  msg_ps/msg_sb per pair [128=n-chunk, (k, q, h')=512]
  agg_sb per pair [128=(q,h'), m=512]
  sifo per pair [128, (i|f|o) m = 1536] fp16, tg [128, 512] fp16
"""

import sys
import types
from contextlib import ExitStack

import numpy as np

import concourse.bass as bass
import concourse.mybir as mybir
import concourse.tile as tile
from concourse import bacc
from concourse.bass_utils import run_bass_kernel_spmd


def _ensure_ntff_hook():
    """Register the axon NTFF profile hook if the image's antenv lacks it."""
    try:
        from antenv import axon_hooks  # noqa: F401
        return
    except ImportError:
        pass
    hook = None
    try:
        import importlib
        tb = importlib.import_module("trn_agent_boot.trn_boot")
        hook = tb._ntff_profile_via_ctypes("/opt/axon/libaxon_pjrt.so")
    except Exception:
        hook = None
    mod = types.ModuleType("antenv.axon_hooks")
    mod._hook = hook
    mod.set_axon_ntff_profile_hook = lambda h: setattr(mod, "_hook", h)
    mod.get_axon_ntff_profile_hook = lambda: mod._hook
    import antenv
    sys.modules["antenv.axon_hooks"] = mod
    antenv.axon_hooks = mod


AF = mybir.ActivationFunctionType
DT = mybir.dt
ALU = mybir.AluOpType

# Problem constants (hardcoded per contract)
B, N, T, F = 32, 512, 168, 8
H = 64
NCORES = 8
BL = B // NCORES          # 4 batches per core
NPAIR = BL // 2           # 2 pairs
KT = N // 128             # 4 station k-tiles
XR = 2 * (F + 1)          # 18 stacked (x, ones) rows per pair
TCH = 8                   # timesteps per x DMA chunk


def build_program(t_steps: int = T, n_devices: int = NCORES):
    nc = bacc.Bacc("TRN2", target_bir_lowering=False, debug=False,
                   num_devices=n_devices)

    fp32 = DT.float32
    f16 = DT.float16

    # DRAM I/O (per-core values supplied via in_maps)
    # xT4 rows: b*9 + f  (b = pair*2 + q, f=8 is the ones/bias row)
    xT4_d = nc.dram_tensor("xT4", [NPAIR * XR, t_steps, N], f16,
                           kind="ExternalInput").ap()
    adjT_d = nc.dram_tensor("adjT", [128, KT * N], f16, kind="ExternalInput").ap()
    wgx2_d = nc.dram_tensor("wgx2", [XR, 128], f16, kind="ExternalInput").ap()
    wghbd_d = nc.dram_tensor("wghbd", [128, 128], f16, kind="ExternalInput").ap()
    wlbd_d = nc.dram_tensor("wlbd", [128, 4 * 128], f16, kind="ExternalInput").ap()
    wr12_d = nc.dram_tensor("wr12", [128, H // 2], f16, kind="ExternalInput").ap()
    br1_d = nc.dram_tensor("br1", [H // 2, 1], fp32, kind="ExternalInput").ap()
    wr2_d = nc.dram_tensor("wr2", [H // 2, 1], f16, kind="ExternalInput").ap()
    br2_d = nc.dram_tensor("br2", [1, 1], fp32, kind="ExternalInput").ap()
    out_d = nc.dram_tensor("out", [1, BL], fp32, kind="ExternalOutput").ap()

    n_chunks = (t_steps + TCH - 1) // TCH

    with tile.TileContext(nc) as tc, ExitStack() as ctx:
        const_pool = ctx.enter_context(tc.tile_pool(name="const", bufs=1))
        state_pool = ctx.enter_context(tc.tile_pool(name="state", bufs=1))
        xc_pool = ctx.enter_context(tc.tile_pool(name="xc", bufs=2))
        msgs_pool = ctx.enter_context(tc.tile_pool(name="msgs", bufs=2))
        aggs_pool = ctx.enter_context(tc.tile_pool(name="aggs", bufs=2))
        act_pool = ctx.enter_context(tc.tile_pool(name="acts", bufs=2))
        tmp_pool = ctx.enter_context(tc.tile_pool(name="tmp", bufs=4))

        pp_msg = [
            ctx.enter_context(tc.tile_pool(name="pp_msg0", bufs=1, space="PSUM")),
            ctx.enter_context(tc.tile_pool(name="pp_msg1", bufs=1, space="PSUM")),
        ]
        pp_agg = [
            ctx.enter_context(tc.tile_pool(name="pp_agg0", bufs=1, space="PSUM")),
            ctx.enter_context(tc.tile_pool(name="pp_agg1", bufs=1, space="PSUM")),
        ]
        pp_gate = ctx.enter_context(tc.tile_pool(name="pp_gate", bufs=1, space="PSUM"))

        # ---- constants ----
        adjT_sb = const_pool.tile([128, KT * N], f16)
        nc.sync.dma_start(adjT_sb[:], adjT_d[:])
        wgx2_sb = const_pool.tile([XR, 128], f16)
        nc.sync.dma_start(wgx2_sb[:], wgx2_d[:])
        wghbd_sb = const_pool.tile([128, 128], f16)
        nc.sync.dma_start(wghbd_sb[:], wghbd_d[:])
        wlbd_sb = const_pool.tile([128, 4 * 128], f16)
        nc.sync.dma_start(wlbd_sb[:], wlbd_d[:])
        wr12_sb = const_pool.tile([128, H // 2], f16)
        nc.sync.dma_start(wr12_sb[:], wr12_d[:])
        br1_sb = const_pool.tile([H // 2, 1], fp32)
        nc.sync.dma_start(br1_sb[:], br1_d[:])
        wr2_sb = const_pool.tile([H // 2, 1], f16)
        nc.sync.dma_start(wr2_sb[:], wr2_d[:])
        br2_sb = const_pool.tile([1, 1], fp32)
        nc.sync.dma_start(br2_sb[:], br2_d[:])

        # ---- persistent state, zero-init ----
        hT2 = state_pool.tile([128, NPAIR * N], f16)
        cT2 = state_pool.tile([128, NPAIR * N], fp32)
        nc.gpsimd.memset(hT2[:], 0.0)
        nc.gpsimd.memset(cT2[:], 0.0)

        # x chunks: per pair tiles [18, TCH*N], double buffered
        xc_tiles = {}

        def load_chunk(ch):
            nt = min(TCH, t_steps - ch * TCH)
            pt = []
            for pair in range(NPAIR):
                xt = xc_pool.tile([XR, TCH * N], f16, tag=f"xc{pair}",
                                  name=f"xc{pair}_{ch}")
                nc.sync.dma_start(
                    xt[:, 0:nt * N].rearrange("p (t n) -> p t n", t=nt, n=N),
                    xT4_d[pair * XR:(pair + 1) * XR, ch * TCH:ch * TCH + nt, :])
                pt.append(xt)
            xc_tiles[ch] = pt

        load_chunk(0)
        if n_chunks > 1:
            load_chunk(1)

        msg_sb = [None, None]

        def emit_m1(pair, t):
            """M1 + relu for (pair, t) -> msg_sb[pair]."""
            ch, off = divmod(t, TCH)
            off *= N
            xc = xc_tiles[ch][pair]
            msg_ps = pp_msg[pair].tile([128, KT * 128], fp32, tag="msg_ps",
                                       name=f"msg_ps{pair}")
            for k in range(KT):
                # per-PSUM-region accumulation group: x then h, consecutive
                nc.tensor.matmul(
                    msg_ps[:, k * 128:(k + 1) * 128],
                    xc[:, off + k * 128: off + (k + 1) * 128],
                    wgx2_sb[:],
                    start=True, stop=False, skip_group_check=True,
                )
                nc.tensor.matmul(
                    msg_ps[:, k * 128:(k + 1) * 128],
                    hT2[:, pair * N + k * 128: pair * N + (k + 1) * 128],
                    wghbd_sb[:],
                    start=False, stop=True, skip_group_check=True,
                )
            ms = msgs_pool.tile([128, KT * 128], f16, tag=f"msg_sb{pair}",
                                name=f"msg_sb{pair}")
            nc.vector.tensor_scalar_max(ms[:], msg_ps[:], 0.0)
            msg_sb[pair] = ms

        # ---- prologue: M1 for both pairs at t=0 ----
        emit_m1(0, 0)
        emit_m1(1, 0)

        for tau in range(2 * t_steps):
            t, pair = divmod(tau, 2)
            # prefetch x chunk two blocks ahead
            if pair == 0 and t % TCH == 0:
                ch = t // TCH
                if ch + 2 < n_chunks and (ch + 2) not in xc_tiles:
                    load_chunk(ch + 2)
                if ch - 1 in xc_tiles:
                    del xc_tiles[ch - 1]

            # ---- M2: aggT = msg^T @ adjT ----
            agg_ps = pp_agg[pair].tile([128, N], fp32, tag="agg_ps",
                                       name=f"agg_ps{pair}")
            for k in range(KT):
                nc.tensor.matmul(
                    agg_ps[:],
                    msg_sb[pair][:, k * 128:(k + 1) * 128],
                    adjT_sb[:, k * N:(k + 1) * N],
                    start=(k == 0), stop=(k == KT - 1),
                )
            agg_sb = aggs_pool.tile([128, N], f16, tag=f"agg_sb{pair}",
                                    name=f"agg_sb{pair}")
            nc.vector.tensor_copy(agg_sb[:], agg_ps[:])

            # ---- M3: gates (i, f, o) -> t_ifo, (g) -> t_g ----
            t_ifo = pp_gate.tile([128, 3 * N], fp32, tag="t_ifo")
            t_g = pp_gate.tile([128, N], fp32, tag="t_g")
            for gi, dst in ((0, t_ifo[:, 0:N]), (1, t_ifo[:, N:2 * N]),
                            (2, t_ifo[:, 2 * N:3 * N]), (3, t_g[:])):
                nc.tensor.matmul(
                    dst, wlbd_sb[:, gi * 128:(gi + 1) * 128], agg_sb[:],
                    start=True, stop=True,
                )

            # ---- activations ----
            tg = act_pool.tile([128, N], f16, tag=f"tg{pair}", name=f"tg{pair}")
            nc.scalar.activation(tg[:], t_g[:], AF.Tanh)
            sifo = act_pool.tile([128, 3 * N], f16, tag=f"sifo{pair}",
                                 name=f"sifo{pair}")
            nc.scalar.activation(sifo[:], t_ifo[:], AF.Sigmoid)

            # ---- LSTM tail ----
            c_sl = cT2[:, pair * N:(pair + 1) * N]
            t2 = tmp_pool.tile([128, N], f16, tag=f"t2{pair}", name=f"t2{pair}")
            nc.gpsimd.tensor_tensor(t2[:], sifo[:, 0:N], tg[:], ALU.mult)
            t1 = tmp_pool.tile([128, N], fp32, tag=f"t1{pair}", name=f"t1{pair}")
            nc.vector.tensor_tensor(t1[:], sifo[:, N:2 * N], c_sl, ALU.mult)
            nc.vector.tensor_tensor(c_sl, t1[:], t2[:], ALU.add)
            tc_ = tmp_pool.tile([128, N], f16, tag=f"tc{pair}", name=f"tc{pair}")
            nc.scalar.activation(tc_[:], c_sl, AF.Tanh)
            nc.vector.tensor_tensor(hT2[:, pair * N:(pair + 1) * N],
                                    sifo[:, 2 * N:3 * N], tc_[:], ALU.mult)

            # ---- M1 for (pair, t+1) ----
            if t + 1 < t_steps:
                emit_m1(pair, t + 1)

        # ---- readout head: feat = h[:, station 0, :] ----
        # (reuse the msg PSUM slots: same tag+shape, slice the corner)
        r1_full = pp_msg[0].tile([128, KT * 128], fp32, tag="msg_ps",
                                 name="r1_full")
        r1_ps = r1_full[0:H // 2, 0:BL]
        for b in range(BL):
            pair, q = divmod(b, 2)
            nc.tensor.matmul(
                r1_ps[:, b:b + 1],
                wr12_sb[q * H:(q + 1) * H, :],
                hT2[q * H:(q + 1) * H, pair * N: pair * N + 1],
                start=True, stop=True,
            )
        r1_sb = tmp_pool.tile([H // 2, BL], f16, tag="r1")
        nc.scalar.activation(r1_sb[:], r1_ps[:], AF.Relu, bias=br1_sb[:])
        r2_full = pp_msg[1].tile([128, KT * 128], fp32, tag="msg_ps",
                                 name="r2_full")
        r2_ps = r2_full[0:1, 0:BL]
        nc.tensor.matmul(r2_ps[:], wr2_sb[:], r1_sb[:], start=True, stop=True)
        out_sb = tmp_pool.tile([1, BL], fp32, tag="out")
        nc.scalar.activation(out_sb[:], r2_ps[:], AF.Identity, bias=br2_sb[:])
        nc.sync.dma_start(out_d[:], out_sb[:])

    nc.compile()
    return nc


def prep_inputs(pollution_seq, adj, Wg, bg, Wl, bl, Wr1, br1, Wr2, br2,
                t_steps: int = T):
    """Host-side prep: shard + relayout. Returns per-core in_maps list."""
    assert np.allclose(bl, 0.0), "kernel folds bl only for bl==0"
    f32, f16 = np.float32, np.float16

    adjT = np.ascontiguousarray(adj.T).astype(f32)          # [n, m]
    adjT_tiled = np.ascontiguousarray(
        adjT.reshape(KT, 128, N).transpose(1, 0, 2).reshape(128, KT * N)
    ).astype(f16)

    wgx = Wg[:F].astype(f32)                                 # [8, 64]
    wgh = Wg[F:].astype(f32)                                 # [64, 64]

    # 2-blockdiag [x-weights; bias] for the two batches of a pair
    wgx2 = np.zeros((XR, 128), f32)
    for q in range(2):
        wgx2[q * (F + 1):q * (F + 1) + F, q * H:(q + 1) * H] = wgx
        wgx2[q * (F + 1) + F, q * H:(q + 1) * H] = bg
    wghbd = np.zeros((128, 128), f32)
    wghbd[0:H, 0:H] = wgh
    wghbd[H:128, H:128] = wgh
    # gate order (i, f, o, g) <- reference (i, f, g, o)
    wlbd = np.zeros((128, 4 * 128), f32)
    for gi, gc in enumerate((0, 1, 3, 2)):
        blk = Wl[:, gc * H:(gc + 1) * H]
        wlbd[0:H, gi * 128:gi * 128 + H] = blk
        wlbd[H:128, gi * 128 + H:(gi + 1) * 128] = blk

    wr12 = np.concatenate([Wr1, Wr1], axis=0)                # [128, 32]

    common = dict(
        adjT=adjT_tiled, wgx2=wgx2.astype(f16), wghbd=wghbd.astype(f16),
        wlbd=wlbd.astype(f16), wr12=wr12.astype(f16),
        br1=br1.reshape(H // 2, 1).astype(f32),
        wr2=Wr2.reshape(H // 2, 1).astype(f16),
        br2=br2.reshape(1, 1).astype(f32),
    )

    in_maps = []
    for i in range(NCORES):
        xc = pollution_seq[i * BL:(i + 1) * BL, :, :t_steps, :]  # [4, 512, t, 8]
        xT = xc.transpose(0, 3, 2, 1)                            # [4, 8, t, 512]
        ones = np.ones((BL, 1, t_steps, N), f32)
        xT4 = np.concatenate([xT.astype(f32), ones], axis=1)     # [4, 9, t, 512]
        xT4 = xT4.reshape(NPAIR * XR, t_steps, N)
        m = dict(common)
        m["xT4"] = np.ascontiguousarray(xT4).astype(f16)
        in_maps.append(m)
    return in_maps


_NC_CACHE = {}


def _get_program(t_steps):
    if t_steps not in _NC_CACHE:
        _NC_CACHE[t_steps] = build_program(t_steps)
    return _NC_CACHE[t_steps]


def kernel(pollution_seq, adj, Wg, bg, Wl, bl, Wr1, br1, Wr2, br2,
           trace=False):
    if trace:
        _ensure_ntff_hook()
    nc = _get_program(T)
    in_maps = prep_inputs(pollution_seq, adj, Wg, bg, Wl, bl, Wr1, br1,
                          Wr2, br2, T)
    res = run_bass_kernel_spmd(nc, in_maps, list(range(NCORES)), trace=trace)
    outs = [res.results[i]["out"].reshape(1, BL) for i in range(NCORES)]
    full = np.concatenate([o.T for o in outs], axis=0).astype(np.float32)  # [32,1]
    if trace:
        kernel.last_exec_time_ns = res.exec_time_ns
        kernel.last_results = res
    return full


# revision 10
# speedup vs baseline: 1.2007x; 1.1116x over previous
"""GC-LSTM (graph-conv LSTM) Trainium2 kernel — v3 (staggered pair pipeline).

Model (per batch b, timestep t):
    msg  = relu([x_t, h] @ Wg + bg)          # [N, H]
    agg  = adj @ msg                         # graph aggregation over stations
    gates= agg @ Wl + bl                     # [N, 4H]
    c    = sig(f)*c + sig(i)*tanh(g)
    h    = sig(o)*tanh(c)
final: out = relu(h[:, 0, :] @ Wr1 + br1) @ Wr2 + br2     # [B, 1]

Sharding: data-parallel over B=32 across 8 cores (BL=4 per core), adj and
weights replicated, h/c device-local through the T=168 scan.

v3 design:
- 4 batches/core processed as 2 pairs; each pair's recurrence chain is a
  software pipeline stage, emitted as alternating half-step blocks
  (p0@t, p1@t, p0@t+1, ...) so one pair's PE work overlaps the other
  pair's activation/LSTM tail.
- Block-diagonal weights put both batches of a pair in one matmul:
    M1x: lhsT = stacked x [18,128], rhs = 2-blockdiag Wgx [18,128]
    M1h: lhsT = hT2 slice [128,128], rhs = 2-blockdiag Wgh [128,128]
    M3:  lhsT = 2-blockdiag Wl_gate [128,128], rhs = agg_sb [128,512]
  (accumulation-group matmuls for one PSUM region stay consecutive).
- Gates per pair: t_ifo [128,1536] PSUM (one full-width sigmoid) and
  t_g [128,512] (one tanh). relu & agg cast on GpSimd; LSTM arithmetic
  on DVE; sig/tanh on ACT.

Layouts:
  hT2/cT2 [128=(q,h), (pair,n)=1024]  (q = batch parity in pair)
  msg_ps/msg_sb per pair [128=n-chunk, (k, q, h')=512]
  agg_sb per pair [128=(q,h'), m=512]
  sifo per pair [128, (i|f|o) m = 1536] fp16, tg [128, 512] fp16
"""

import sys
import types
from contextlib import ExitStack

import numpy as np

import concourse.bass as bass
import concourse.mybir as mybir
import concourse.tile as tile
from concourse import bacc
from concourse.bass_utils import run_bass_kernel_spmd


def _ensure_ntff_hook():
    """Register the axon NTFF profile hook if the image's antenv lacks it."""
    try:
        from antenv import axon_hooks  # noqa: F401
        return
    except ImportError:
        pass
    hook = None
    try:
        import importlib
        tb = importlib.import_module("trn_agent_boot.trn_boot")
        hook = tb._ntff_profile_via_ctypes("/opt/axon/libaxon_pjrt.so")
    except Exception:
        hook = None
    mod = types.ModuleType("antenv.axon_hooks")
    mod._hook = hook
    mod.set_axon_ntff_profile_hook = lambda h: setattr(mod, "_hook", h)
    mod.get_axon_ntff_profile_hook = lambda: mod._hook
    import antenv
    sys.modules["antenv.axon_hooks"] = mod
    antenv.axon_hooks = mod


AF = mybir.ActivationFunctionType
DT = mybir.dt
ALU = mybir.AluOpType

# Problem constants (hardcoded per contract)
B, N, T, F = 32, 512, 168, 8
H = 64
NCORES = 8
BL = B // NCORES          # 4 batches per core
NPAIR = BL // 2           # 2 pairs
KT = N // 128             # 4 station k-tiles
XR = 2 * (F + 1)          # 18 stacked (x, ones) rows per pair
TCH = 8                   # timesteps per x DMA chunk


def build_program(t_steps: int = T, n_devices: int = NCORES):
    nc = bacc.Bacc("TRN2", target_bir_lowering=False, debug=False,
                   num_devices=n_devices)

    fp32 = DT.float32
    f16 = DT.float16

    # DRAM I/O (per-core values supplied via in_maps)
    # xT4 rows: b*9 + f  (b = pair*2 + q, f=8 is the ones/bias row)
    xT4_d = nc.dram_tensor("xT4", [NPAIR * XR, t_steps, N], f16,
                           kind="ExternalInput").ap()
    adjT_d = nc.dram_tensor("adjT", [128, KT * N], f16, kind="ExternalInput").ap()
    wgx2_d = nc.dram_tensor("wgx2", [XR, 128], f16, kind="ExternalInput").ap()
    wghbd_d = nc.dram_tensor("wghbd", [128, 128], f16, kind="ExternalInput").ap()
    wlbd_d = nc.dram_tensor("wlbd", [128, 4 * 128], f16, kind="ExternalInput").ap()
    wr12_d = nc.dram_tensor("wr12", [128, H // 2], f16, kind="ExternalInput").ap()
    br1_d = nc.dram_tensor("br1", [H // 2, 1], fp32, kind="ExternalInput").ap()
    wr2_d = nc.dram_tensor("wr2", [H // 2, 1], f16, kind="ExternalInput").ap()
    br2_d = nc.dram_tensor("br2", [1, 1], fp32, kind="ExternalInput").ap()
    out_d = nc.dram_tensor("out", [1, BL], fp32, kind="ExternalOutput").ap()

    n_chunks = (t_steps + TCH - 1) // TCH

    with tile.TileContext(nc) as tc, ExitStack() as ctx:
        const_pool = ctx.enter_context(tc.tile_pool(name="const", bufs=1))
        state_pool = ctx.enter_context(tc.tile_pool(name="state", bufs=1))
        xc_pool = ctx.enter_context(tc.tile_pool(name="xc", bufs=2))
        msgs_pool = ctx.enter_context(tc.tile_pool(name="msgs", bufs=2))
        aggs_pool = ctx.enter_context(tc.tile_pool(name="aggs", bufs=2))
        act_pool = ctx.enter_context(tc.tile_pool(name="acts", bufs=2))
        tmp_pool = ctx.enter_context(tc.tile_pool(name="tmp", bufs=4))

        pp_msg = [
            ctx.enter_context(tc.tile_pool(name="pp_msg0", bufs=1, space="PSUM")),
            ctx.enter_context(tc.tile_pool(name="pp_msg1", bufs=1, space="PSUM")),
        ]
        pp_agg = [
            ctx.enter_context(tc.tile_pool(name="pp_agg0", bufs=1, space="PSUM")),
            ctx.enter_context(tc.tile_pool(name="pp_agg1", bufs=1, space="PSUM")),
        ]
        pp_gate = ctx.enter_context(tc.tile_pool(name="pp_gate", bufs=1, space="PSUM"))

        # ---- constants ----
        adjT_sb = const_pool.tile([128, KT * N], f16)
        nc.sync.dma_start(adjT_sb[:], adjT_d[:])
        wgx2_sb = const_pool.tile([XR, 128], f16)
        nc.sync.dma_start(wgx2_sb[:], wgx2_d[:])
        wghbd_sb = const_pool.tile([128, 128], f16)
        nc.sync.dma_start(wghbd_sb[:], wghbd_d[:])
        wlbd_sb = const_pool.tile([128, 4 * 128], f16)
        nc.sync.dma_start(wlbd_sb[:], wlbd_d[:])
        wr12_sb = const_pool.tile([128, H // 2], f16)
        nc.sync.dma_start(wr12_sb[:], wr12_d[:])
        br1_sb = const_pool.tile([H // 2, 1], fp32)
        nc.sync.dma_start(br1_sb[:], br1_d[:])
        wr2_sb = const_pool.tile([H // 2, 1], f16)
        nc.sync.dma_start(wr2_sb[:], wr2_d[:])
        br2_sb = const_pool.tile([1, 1], fp32)
        nc.sync.dma_start(br2_sb[:], br2_d[:])

        # ---- persistent state, zero-init ----
        hT2 = state_pool.tile([128, NPAIR * N], f16)
        cT2 = state_pool.tile([128, NPAIR * N], fp32)
        nc.gpsimd.memset(hT2[:], 0.0)
        nc.gpsimd.memset(cT2[:], 0.0)

        # x chunks: per pair tiles [18, TCH*N], double buffered
        xc_tiles = {}

        def load_chunk(ch):
            nt = min(TCH, t_steps - ch * TCH)
            pt = []
            for pair in range(NPAIR):
                xt = xc_pool.tile([XR, TCH * N], f16, tag=f"xc{pair}",
                                  name=f"xc{pair}_{ch}")
                nc.sync.dma_start(
                    xt[:, 0:nt * N].rearrange("p (t n) -> p t n", t=nt, n=N),
                    xT4_d[pair * XR:(pair + 1) * XR, ch * TCH:ch * TCH + nt, :])
                pt.append(xt)
            xc_tiles[ch] = pt

        load_chunk(0)
        if n_chunks > 1:
            load_chunk(1)

        msg_sb = [None, None]
        msg_ps_t = [None, None]

        def emit_m1_mm(pair, t):
            """M1 matmuls for (pair, t) -> msg_ps_t[pair]."""
            ch, off = divmod(t, TCH)
            off *= N
            xc = xc_tiles[ch][pair]
            msg_ps = pp_msg[pair].tile([128, KT * 128], fp32, tag="msg_ps",
                                       name=f"msg_ps{pair}")
            for k in range(KT):
                # per-PSUM-region accumulation group: x then h, consecutive
                nc.tensor.matmul(
                    msg_ps[:, k * 128:(k + 1) * 128],
                    xc[:, off + k * 128: off + (k + 1) * 128],
                    wgx2_sb[:],
                    start=True, stop=False, skip_group_check=True,
                )
                nc.tensor.matmul(
                    msg_ps[:, k * 128:(k + 1) * 128],
                    hT2[:, pair * N + k * 128: pair * N + (k + 1) * 128],
                    wghbd_sb[:],
                    start=False, stop=True, skip_group_check=True,
                )
            msg_ps_t[pair] = msg_ps

        def emit_relu(pair):
            ms = msgs_pool.tile([128, KT * 128], f16, tag=f"msg_sb{pair}",
                                name=f"msg_sb{pair}")
            nc.vector.tensor_scalar_max(ms[:], msg_ps_t[pair][:], 0.0)
            msg_sb[pair] = ms

        # ---- prologue: M1 for pair0 at t=0 ----
        emit_m1_mm(0, 0)
        emit_relu(0)

        for tau in range(2 * t_steps):
            t, pair = divmod(tau, 2)
            other = 1 - pair
            # (pair, t) of the NEXT block; its M1 is emitted in this block
            # because its h-dependency resolved two blocks ago.
            nxt_t = t + (1 if pair == 1 else 0)
            # prefetch x chunk two blocks ahead
            if pair == 0 and t % TCH == 0:
                ch = t // TCH
                if ch + 2 < n_chunks and (ch + 2) not in xc_tiles:
                    load_chunk(ch + 2)
                if ch - 1 in xc_tiles:
                    del xc_tiles[ch - 1]

            # ---- M2: aggT = msg^T @ adjT ----
            agg_ps = pp_agg[pair].tile([128, N], fp32, tag="agg_ps",
                                       name=f"agg_ps{pair}")
            for k in range(KT):
                nc.tensor.matmul(
                    agg_ps[:],
                    msg_sb[pair][:, k * 128:(k + 1) * 128],
                    adjT_sb[:, k * N:(k + 1) * N],
                    start=(k == 0), stop=(k == KT - 1),
                )
            agg_sb = aggs_pool.tile([128, N], f16, tag=f"agg_sb{pair}",
                                    name=f"agg_sb{pair}")
            nc.vector.tensor_copy(agg_sb[:], agg_ps[:])

            # ---- M3: gate order (g, f, i, o); wlbd blocks are (i,f,o,g) ----
            t_fio = pp_gate.tile([128, 3 * N], fp32, tag="t_fio")
            t_g = pp_gate.tile([128, N], fp32, tag="t_g")
            for gi, dst in ((3, t_g[:]), (1, t_fio[:, 0:N]),
                            (0, t_fio[:, N:2 * N]), (2, t_fio[:, 2 * N:3 * N])):
                nc.tensor.matmul(
                    dst, wlbd_sb[:, gi * 128:(gi + 1) * 128], agg_sb[:],
                    start=True, stop=True,
                )

            # ---- M1 matmuls for the next block's pair-step ----
            if nxt_t < t_steps:
                emit_m1_mm(other, nxt_t)

            # ---- activations: tanh(g), then sig f/i/o split for early tail ----
            tg = act_pool.tile([128, N], f16, tag=f"tg{pair}", name=f"tg{pair}")
            nc.scalar.activation(tg[:], t_g[:], AF.Tanh)
            sfio = act_pool.tile([128, 3 * N], f16, tag=f"sfio{pair}",
                                 name=f"sfio{pair}")
            nc.scalar.activation(sfio[:, 0:N], t_fio[:, 0:N], AF.Sigmoid)
            nc.scalar.activation(sfio[:, N:2 * N], t_fio[:, N:2 * N], AF.Sigmoid)
            nc.scalar.activation(sfio[:, 2 * N:3 * N], t_fio[:, 2 * N:3 * N],
                                 AF.Sigmoid)

            # ---- LSTM tail (DVE) + relu for next block interleaved ----
            c_sl = cT2[:, pair * N:(pair + 1) * N]
            t1 = tmp_pool.tile([128, N], fp32, tag=f"t1{pair}", name=f"t1{pair}")
            nc.vector.tensor_tensor(t1[:], sfio[:, 0:N], c_sl, ALU.mult)
            t2 = tmp_pool.tile([128, N], f16, tag=f"t2{pair}", name=f"t2{pair}")
            nc.vector.tensor_tensor(t2[:], sfio[:, N:2 * N], tg[:], ALU.mult)
            nc.vector.tensor_tensor(c_sl, t1[:], t2[:], ALU.add)
            tc_ = tmp_pool.tile([128, N], f16, tag=f"tc{pair}", name=f"tc{pair}")
            nc.scalar.activation(tc_[:], c_sl, AF.Tanh)
            if nxt_t < t_steps:
                emit_relu(other)
            nc.vector.tensor_tensor(hT2[:, pair * N:(pair + 1) * N],
                                    sfio[:, 2 * N:3 * N], tc_[:], ALU.mult)

        # ---- readout head: feat = h[:, station 0, :] ----
        # (reuse the msg PSUM slots: same tag+shape, slice the corner)
        r1_full = pp_msg[0].tile([128, KT * 128], fp32, tag="msg_ps",
                                 name="r1_full")
        r1_ps = r1_full[0:H // 2, 0:BL]
        for b in range(BL):
            pair, q = divmod(b, 2)
            nc.tensor.matmul(
                r1_ps[:, b:b + 1],
                wr12_sb[q * H:(q + 1) * H, :],
                hT2[q * H:(q + 1) * H, pair * N: pair * N + 1],
                start=True, stop=True,
            )
        r1_sb = tmp_pool.tile([H // 2, BL], f16, tag="r1")
        nc.scalar.activation(r1_sb[:], r1_ps[:], AF.Relu, bias=br1_sb[:])
        r2_full = pp_msg[1].tile([128, KT * 128], fp32, tag="msg_ps",
                                 name="r2_full")
        r2_ps = r2_full[0:1, 0:BL]
        nc.tensor.matmul(r2_ps[:], wr2_sb[:], r1_sb[:], start=True, stop=True)
        out_sb = tmp_pool.tile([1, BL], fp32, tag="out")
        nc.scalar.activation(out_sb[:], r2_ps[:], AF.Identity, bias=br2_sb[:])
        nc.sync.dma_start(out_d[:], out_sb[:])

    nc.compile()
    return nc


def prep_inputs(pollution_seq, adj, Wg, bg, Wl, bl, Wr1, br1, Wr2, br2,
                t_steps: int = T):
    """Host-side prep: shard + relayout. Returns per-core in_maps list."""
    assert np.allclose(bl, 0.0), "kernel folds bl only for bl==0"
    f32, f16 = np.float32, np.float16

    adjT = np.ascontiguousarray(adj.T).astype(f32)          # [n, m]
    adjT_tiled = np.ascontiguousarray(
        adjT.reshape(KT, 128, N).transpose(1, 0, 2).reshape(128, KT * N)
    ).astype(f16)

    wgx = Wg[:F].astype(f32)                                 # [8, 64]
    wgh = Wg[F:].astype(f32)                                 # [64, 64]

    # 2-blockdiag [x-weights; bias] for the two batches of a pair
    wgx2 = np.zeros((XR, 128), f32)
    for q in range(2):
        wgx2[q * (F + 1):q * (F + 1) + F, q * H:(q + 1) * H] = wgx
        wgx2[q * (F + 1) + F, q * H:(q + 1) * H] = bg
    wghbd = np.zeros((128, 128), f32)
    wghbd[0:H, 0:H] = wgh
    wghbd[H:128, H:128] = wgh
    # gate order (i, f, o, g) <- reference (i, f, g, o)
    wlbd = np.zeros((128, 4 * 128), f32)
    for gi, gc in enumerate((0, 1, 3, 2)):
        blk = Wl[:, gc * H:(gc + 1) * H]
        wlbd[0:H, gi * 128:gi * 128 + H] = blk
        wlbd[H:128, gi * 128 + H:(gi + 1) * 128] = blk

    wr12 = np.concatenate([Wr1, Wr1], axis=0)                # [128, 32]

    common = dict(
        adjT=adjT_tiled, wgx2=wgx2.astype(f16), wghbd=wghbd.astype(f16),
        wlbd=wlbd.astype(f16), wr12=wr12.astype(f16),
        br1=br1.reshape(H // 2, 1).astype(f32),
        wr2=Wr2.reshape(H // 2, 1).astype(f16),
        br2=br2.reshape(1, 1).astype(f32),
    )

    in_maps = []
    for i in range(NCORES):
        xc = pollution_seq[i * BL:(i + 1) * BL, :, :t_steps, :]  # [4, 512, t, 8]
        xT = xc.transpose(0, 3, 2, 1)                            # [4, 8, t, 512]
        ones = np.ones((BL, 1, t_steps, N), f32)
        xT4 = np.concatenate([xT.astype(f32), ones], axis=1)     # [4, 9, t, 512]
        xT4 = xT4.reshape(NPAIR * XR, t_steps, N)
        m = dict(common)
        m["xT4"] = np.ascontiguousarray(xT4).astype(f16)
        in_maps.append(m)
    return in_maps


_NC_CACHE = {}


def _get_program(t_steps):
    if t_steps not in _NC_CACHE:
        _NC_CACHE[t_steps] = build_program(t_steps)
    return _NC_CACHE[t_steps]


def kernel(pollution_seq, adj, Wg, bg, Wl, bl, Wr1, br1, Wr2, br2,
           trace=False):
    if trace:
        _ensure_ntff_hook()
    nc = _get_program(T)
    in_maps = prep_inputs(pollution_seq, adj, Wg, bg, Wl, bl, Wr1, br1,
                          Wr2, br2, T)
    res = run_bass_kernel_spmd(nc, in_maps, list(range(NCORES)), trace=trace)
    outs = [res.results[i]["out"].reshape(1, BL) for i in range(NCORES)]
    full = np.concatenate([o.T for o in outs], axis=0).astype(np.float32)  # [32,1]
    if trace:
        kernel.last_exec_time_ns = res.exec_time_ns
        kernel.last_results = res
    return full
